# revision 17
# baseline (speedup 1.0000x reference)
"""Trainium2 Bass kernel for nn_GroupLinear: channel-shuffled grouped MLP.

Computes, for x [4096, 16384]:
    h = relu(einsum('bgi,gio->bgo', x[:, perm1].reshape(B,128,128), W1) + b1)
    h = relu(einsum('bgi,gio->bgo', h.reshape(B,8192)[:, perm2].reshape(B,128,64), W2) + b2)
    y = sigmoid(((h.reshape(B,4096) @ W3 + b3) @ W4 + b4) @ W5 + b5)

Sharding: data-parallel over batch across 8 cores (512 rows/core); weights
replicated. Host-side layout prep (same category as the baseline's W3@W4@W5
collapse and weight/index repacking): the x shard is uploaded transposed in
bf16 (channel-major [16384, nb]), so the perm1 shuffle IS the first read of
x — a non-transpose DRAM-source dma_gather at ~1 TB/s. No separate load or
on-device transpose phase exists.

Per-core pipeline (full batch nb=512):
  B) GL1: non-transpose DRAM gather of perm1 rows out of the xt input
     (channel-major direct) -> paired 64-wide matmuls vs W1 packed 2/PSUM
     bank -> relu+bias -> 2-pair staging tile -> HWDGE write (SP ring)
     into h1d DRAM [8192, nb] bf16.
  C) GL2: non-transpose DRAM gather of perm2 rows from h1d -> block-diag W2
     pair matmuls (4 groups/PSUM tile) -> relu+bias -> h2 tiles in SBUF.
  D) head: 32 accumulating matmuls vs collapsed W3@W4@W5 -> sigmoid -> y.

Total device DMA per rep: 16.8 (gather1) + 8.4 (h1 write) + 8.4 (gather2)
= 33.6 MB. Transpose-mode / SBUF-source gathers are avoided entirely
(measured ~60-100 GB/s on this part vs ~1 TB/s for non-transpose DRAM
gathers). h1d is double-buffered so rep r+1's writes overlap rep r's
gathers.
"""

import hashlib

import numpy as np

import concourse.bass as bass
import concourse.mybir as mybir
import concourse.tile as tile
from concourse import bacc, bass_utils, library_config

G = 128          # groups
C1 = G * 128     # 16384 input channels
C2 = G * 64      # 8192 channels after GL1
N_CORES = 8
B_FULL = 4096

F32 = mybir.dt.float32
BF16 = mybir.dt.bfloat16
I16 = mybir.dt.int16
FP8 = mybir.dt.float8e4

K1 = 2048        # perm1 idxs per dma_gather call
K2 = 2048        # perm2 idxs per dma_gather call


def build_nc(nb: int, b345: float, reps: int = 1, mid_bf16: int = 1,
             cblk: int = 1024, ablate: int = 0):
    """Build the per-core Bass program for batch-per-core nb.

    reps > 1 repeats the whole pipeline inside one NEFF (for timing:
    device time per rep = (T_reps - T_1) / (reps - 1), cancelling the
    fixed per-execution dispatch overhead).
    """
    nc = bacc.Bacc("TRN2", debug=False)
    xt = nc.dram_tensor("xt", [C1, nb], BF16, kind="ExternalInput")
    w1 = nc.dram_tensor("w1", [128, G * 64], BF16, kind="ExternalInput")
    w2 = nc.dram_tensor("w2", [128, 64 * 64], BF16, kind="ExternalInput")
    w345 = nc.dram_tensor("w345", [128, 32], BF16, kind="ExternalInput")
    b1p = nc.dram_tensor("b1p", [128, 64], F32, kind="ExternalInput")
    b2q = nc.dram_tensor("b2q", [128, 32], F32, kind="ExternalInput")
    p1g = nc.dram_tensor("p1g", [128, C1 // 16], I16, kind="ExternalInput")
    p2g = nc.dram_tensor("p2g", [128, C2 // 16], I16, kind="ExternalInput")
    y = nc.dram_tensor("y", [1, nb], F32, kind="ExternalOutput")

    relu_t = mybir.ActivationFunctionType.Relu
    sigm_t = mybir.ActivationFunctionType.Sigmoid

    with tile.TileContext(nc) as tc:
        with (
            tc.tile_pool(name="const", bufs=1) as cpool,
            tc.tile_pool(name="h2p", bufs=1) as h2pool,
            tc.tile_pool(name="work", bufs=2) as pool,
            tc.tile_pool(name="psum", bufs=2, space="PSUM") as psum,
            tc.tile_pool(name="dram", bufs=1, space="DRAM") as dpool,
        ):
            # ---- constants / weights preload ----
            w1s = cpool.tile([128, G * 64], BF16)
            nc.sync.dma_start(w1s[:], w1.ap())
            w2s = cpool.tile([128, 64 * 64], BF16)
            nc.sync.dma_start(w2s[:], w2.ap())
            w345s = cpool.tile([128, 32], BF16)
            nc.sync.dma_start(w345s[:], w345.ap())
            b1s = cpool.tile([128, 64], F32)
            nc.sync.dma_start(b1s[:], b1p.ap())
            b2s = cpool.tile([128, 32], F32)
            nc.sync.dma_start(b2s[:], b2q.ap())
            p1s = cpool.tile([128, C1 // 16], I16)
            nc.sync.dma_start(p1s[:], p1g.ap())
            p2s = cpool.tile([128, C2 // 16], I16)
            nc.sync.dma_start(p2s[:], p2g.ap())

            nc.gpsimd.load_library(library_config.mlp)

            for _rep in range(reps):
                run_rep(nc, tc, pool, psum, h2pool, dpool, xt, y,
                        w1s, w2s, w345s, b1s, b2s, p1s, p2s,
                        nb, b345, relu_t, sigm_t, ablate)

    nc.compile()
    return nc


def run_rep(nc, tc, pool, psum, h2pool, dpool, xt, y,
            w1s, w2s, w345s, b1s, b2s, p1s, p2s,
            nb, b345, relu_t, sigm_t, ablate=0):
    h1d = dpool.tile([C2, nb], BF16, tag="h1d", name="h1d", bufs=2)

    # ---- phase B: GL1 (DRAM gather of xt + paired matmuls -> h1d) ----
    for k in range(C1 // K1):
        g1 = pool.tile([128, K1 // 128, nb], BF16, tag="g1", name="g1", bufs=3)
        nc.gpsimd.dma_gather(
            g1[:], xt.ap(), p1s[:, k * (K1 // 16):(k + 1) * (K1 // 16)],
            K1, K1, nb, single_packet=False)
        if ablate == 3:
            continue
        h1st = None
        for j in range(K1 // 256):          # pairs of groups
            pair = (K1 // 256) * k + j
            ps_b = psum.tile([128, nb], F32, tag="ps_b")
            for half in range(2):
                blk = 2 * j + half
                g = (K1 // 128) * k + blk
                nc.tensor.matmul(
                    ps_b[64 * half:64 * (half + 1), :],
                    lhsT=w1s[:, g * 64:(g + 1) * 64], rhs=g1[:, blk, :],
                    start=True, stop=True,
                    tile_position=(0, 64 * half) if half else None)
            if j % 2 == 0:
                h1st = pool.tile([128, 2 * nb], BF16, tag="h1st", bufs=3,
                                 name="h1st")
            nc.scalar.activation(h1st[:, (j % 2) * nb:(j % 2 + 1) * nb],
                                 ps_b[:], relu_t,
                                 bias=b1s[:, pair:pair + 1])
            if j % 2 == 1:
                ch0 = (pair - 1) * 128
                nc.sync.dma_start(
                    h1d[ch0:ch0 + 256, :].rearrange(
                        "(c p) b -> p c b", c=2), h1st[:])

    if ablate == 3:
        yt0 = pool.tile([1, nb], F32, tag="yt", bufs=2, name="yt0")
        nc.vector.tensor_copy(yt0[:], g1[0:1, 0, :])
        nc.sync.dma_start(y.ap(), yt0[:])
        return
    if ablate == 4:
        rb = pool.tile([1, nb], BF16, tag="rb", bufs=2, name="rb")
        nc.sync.dma_start(rb[:], h1d[C2 - 1:C2, :])
        yt0 = pool.tile([1, nb], F32, tag="yt", bufs=2, name="yt0")
        nc.vector.tensor_copy(yt0[:], rb[:])
        nc.sync.dma_start(y.ap(), yt0[:])
        return

    # ---- phase C: GL2 (DRAM gather + block-diag matmuls) ----
    h2 = []
    for k in range(C2 // K2):
        g2 = pool.tile([128, K2 // 128, nb], BF16, tag="g2", name="g2")
        nc.gpsimd.dma_gather(
            g2[:], h1d[:], p2s[:, k * (K2 // 16):(k + 1) * (K2 // 16)],
            K2, K2, nb, single_packet=False)
        for j in range(K2 // 256):          # quads of groups
            t = (K2 // 256) * k + j
            ps_c = psum.tile([128, nb], F32, tag="ps_b", bufs=2)
            for half in range(2):
                blk = 2 * j + half
                q = (K2 // 128) * k + blk
                nc.tensor.matmul(
                    ps_c[64 * half:64 * (half + 1), :],
                    lhsT=w2s[:, q * 64:(q + 1) * 64], rhs=g2[:, blk, :],
                    start=True, stop=True,
                    tile_position=(0, 64 * half) if half else None)
            h2t = h2pool.tile([128, nb], BF16, tag=f"h2_{t}", name=f"h2_{t}")
            nc.scalar.activation(h2t[:], ps_c[:], relu_t,
                                 bias=b2s[:, t:t + 1])
            h2.append(h2t)

    # ---- phase D: head ----
    ps_d = psum.tile([1, nb], F32, tag="ps_d", bufs=1)
    for t in range(32):
        nc.tensor.matmul(ps_d[:], lhsT=w345s[:, t:t + 1], rhs=h2[t][:],
                         start=(t == 0), stop=(t == 31))
    yt = pool.tile([1, nb], F32, tag="yt", bufs=2)
    nc.scalar.activation(yt[:], ps_d[:], sigm_t, bias=float(b345))
    nc.sync.dma_start(y.ap(), yt[:])


def _gather_table(perm: np.ndarray, chunk: int) -> np.ndarray:
    """Index table for dma_gather: chunk c occupies cols [c*chunk/16,
    (c+1)*chunk/16); within a chunk, idx position i = col*16 + row.
    Rows 0-15 hold the indices; replicated to all 128 partitions."""
    n = perm.shape[0]
    cols = chunk // 16
    t = np.zeros((128, (n // chunk) * cols), dtype=np.int16)
    for c in range(n // chunk):
        blk = perm[c * chunk:(c + 1) * chunk].reshape(cols, 16).T  # [16, cols]
        t[:, c * cols:(c + 1) * cols] = np.tile(blk, (8, 1))
    return t


def prep_host(perm1, perm2, W1, b1, W2, b2, W3, b3, W4, b4, W5, b5,
              mid_bf16=1):
    """Host-side layout prep of weights / index tables (replicated per core)."""
    import ml_dtypes
    wdt = ml_dtypes.bfloat16
    w1h = np.ascontiguousarray(
        W1.astype(np.float32).transpose(1, 0, 2).reshape(128, G * 64)).astype(wdt)
    w2h = np.zeros((128, 64 * 64), dtype=wdt)
    for q in range(64):
        w2h[0:64, q * 64:q * 64 + 32] = W2[2 * q].astype(wdt)
        w2h[64:128, q * 64 + 32:(q + 1) * 64] = W2[2 * q + 1].astype(wdt)
    wv = (W3.astype(np.float64) @ W4.astype(np.float64) @ W5.astype(np.float64))
    w345h = np.ascontiguousarray(
        wv.astype(np.float32).reshape(32, 128).T).astype(wdt)
    b345 = float(
        (b3.astype(np.float64) @ W4.astype(np.float64) @ W5.astype(np.float64)
         + b4.astype(np.float64) @ W5.astype(np.float64)
         + b5.astype(np.float64)).reshape(()))
    b1h = np.ascontiguousarray(
        b1.astype(np.float32).reshape(64, 128).T)   # col k = [b1[2k]; b1[2k+1]]
    b2h = np.ascontiguousarray(
        b2.astype(np.float32).reshape(32, 128).T)   # col j = b2[4j:4j+4] stacked
    p1h = _gather_table(perm1.astype(np.int64), K1)
    p2h = _gather_table(perm2.astype(np.int64), K2)
    return {"w1": w1h, "w2": w2h, "w345": w345h, "b1p": b1h, "b2q": b2h,
            "p1g": p1h, "p2g": p2h}, b345


def prep_x(x_shard: np.ndarray) -> np.ndarray:
    """Host layout prep of the x shard: bf16, channel-major [C1, nb]."""
    import ml_dtypes
    return np.ascontiguousarray(
        x_shard.astype(np.float32).astype(ml_dtypes.bfloat16).T)


_NC_CACHE: dict = {}

MID_BF16 = 1


def get_nc(nb: int, b345: float, key_bytes: bytes, mid_bf16: int = MID_BF16):
    key = (nb, mid_bf16,
           hashlib.sha256(key_bytes + np.float64(b345).tobytes()).hexdigest())
    if key not in _NC_CACHE:
        _NC_CACHE[key] = build_nc(nb, b345, mid_bf16=mid_bf16)
    return _NC_CACHE[key]


def kernel(x, perm1, perm2, W1, b1, W2, b2, W3, b3, W4, b4, W5, b5):
    x = np.asarray(x)
    consts, b345 = prep_host(np.asarray(perm1), np.asarray(perm2),
                             np.asarray(W1), np.asarray(b1), np.asarray(W2),
                             np.asarray(b2), np.asarray(W3), np.asarray(b3),
                             np.asarray(W4), np.asarray(b4), np.asarray(W5),
                             np.asarray(b5), mid_bf16=MID_BF16)
    nb = x.shape[0] // N_CORES
    key_bytes = np.asarray(perm1).tobytes() + np.asarray(perm2).tobytes()
    nc = get_nc(nb, b345, key_bytes)
    in_maps = []
    for c in range(N_CORES):
        m = dict(consts)
        m["xt"] = prep_x(x[c * nb:(c + 1) * nb])
        in_maps.append(m)
    res = bass_utils.run_bass_kernel_spmd(nc, in_maps, core_ids=list(range(N_CORES)))
    out = np.concatenate([res.results[c]["y"].reshape(nb) for c in range(N_CORES)])
    return out.reshape(-1, 1).astype(np.float32)


# revision 19
# speedup vs baseline: 3.0894x; 3.0894x over previous
"""Trainium2 Bass kernel for nn_GroupLinear: channel-shuffled grouped MLP.

Computes, for x [4096, 16384]:
    h = relu(einsum('bgi,gio->bgo', x[:, perm1].reshape(B,128,128), W1) + b1)
    h = relu(einsum('bgi,gio->bgo', h.reshape(B,8192)[:, perm2].reshape(B,128,64), W2) + b2)
    y = sigmoid(((h.reshape(B,4096) @ W3 + b3) @ W4 + b4) @ W5 + b5)

Sharding: data-parallel over batch across 8 cores (512 rows/core); weights
replicated. Host-side layout prep (same category as the baseline's W3@W4@W5
collapse and weight/index repacking): the x shard is uploaded transposed in
bf16 (channel-major [16384, nb]), so the perm1 shuffle IS the first read of
x — a non-transpose DRAM-source dma_gather at ~1 TB/s. No separate load or
on-device transpose phase exists.

Per-core pipeline (full batch nb=512):
  B) GL1: non-transpose DRAM gather of perm1 rows out of the xt input
     (channel-major direct) -> paired 64-wide matmuls vs W1 packed 2/PSUM
     bank -> relu+bias -> 2-pair staging tile -> HWDGE write (SP ring)
     into h1d DRAM [8192, nb] bf16.
  C) GL2: non-transpose DRAM gather of perm2 rows from h1d -> block-diag W2
     pair matmuls (4 groups/PSUM tile) -> relu+bias -> h2 tiles in SBUF.
  D) head: 32 accumulating matmuls vs collapsed W3@W4@W5 -> sigmoid -> y.

Total device DMA per rep: 16.8 (gather1) + 8.4 (h1 write) + 8.4 (gather2)
= 33.6 MB. Transpose-mode / SBUF-source gathers are avoided entirely
(measured ~60-100 GB/s on this part vs ~1 TB/s for non-transpose DRAM
gathers). h1d is double-buffered so rep r+1's writes overlap rep r's
gathers.
"""

import hashlib

import numpy as np

import concourse.bass as bass
import concourse.mybir as mybir
import concourse.tile as tile
from concourse import bacc, bass_utils, library_config

G = 128          # groups
C1 = G * 128     # 16384 input channels
C2 = G * 64      # 8192 channels after GL1
N_CORES = 8
B_FULL = 4096

F32 = mybir.dt.float32
BF16 = mybir.dt.bfloat16
I16 = mybir.dt.int16
FP8 = mybir.dt.float8e4

K1 = 4096        # perm1 idxs per dma_gather call
K2 = 2048        # perm2 idxs per dma_gather call


def build_nc(nb: int, b345: float, reps: int = 1, mid_bf16: int = 1,
             cblk: int = 1024, ablate: int = 0):
    """Build the per-core Bass program for batch-per-core nb.

    reps > 1 repeats the whole pipeline inside one NEFF (for timing:
    device time per rep = (T_reps - T_1) / (reps - 1), cancelling the
    fixed per-execution dispatch overhead).
    """
    nc = bacc.Bacc("TRN2", debug=False, num_swdge_queues=4)
    xt = nc.dram_tensor("xt", [C1, nb], BF16, kind="ExternalInput")
    w1 = nc.dram_tensor("w1", [128, G * 64], BF16, kind="ExternalInput")
    w2 = nc.dram_tensor("w2", [128, 64 * 64], BF16, kind="ExternalInput")
    w345 = nc.dram_tensor("w345", [128, 32], BF16, kind="ExternalInput")
    b1p = nc.dram_tensor("b1p", [128, 64], F32, kind="ExternalInput")
    b2q = nc.dram_tensor("b2q", [128, 32], F32, kind="ExternalInput")
    p1g = nc.dram_tensor("p1g", [128, C1 // 16], I16, kind="ExternalInput")
    p2g = nc.dram_tensor("p2g", [128, C2 // 16], I16, kind="ExternalInput")
    y = nc.dram_tensor("y", [1, nb], F32, kind="ExternalOutput")

    relu_t = mybir.ActivationFunctionType.Relu
    sigm_t = mybir.ActivationFunctionType.Sigmoid

    with tile.TileContext(nc) as tc:
        with (
            tc.tile_pool(name="const", bufs=1) as cpool,
            tc.tile_pool(name="h2p", bufs=1) as h2pool,
            tc.tile_pool(name="work", bufs=2) as pool,
            tc.tile_pool(name="psum", bufs=2, space="PSUM") as psum,
            tc.tile_pool(name="dram", bufs=1, space="DRAM") as dpool,
        ):
            # ---- constants / weights preload ----
            w1s = cpool.tile([128, G * 64], BF16)
            nc.sync.dma_start(w1s[:], w1.ap())
            w2s = cpool.tile([128, 64 * 64], BF16)
            nc.sync.dma_start(w2s[:], w2.ap())
            w345s = cpool.tile([128, 32], BF16)
            nc.sync.dma_start(w345s[:], w345.ap())
            b1s = cpool.tile([128, 64], F32)
            nc.sync.dma_start(b1s[:], b1p.ap())
            b2s = cpool.tile([128, 32], F32)
            nc.sync.dma_start(b2s[:], b2q.ap())
            p1s = cpool.tile([128, C1 // 16], I16)
            nc.sync.dma_start(p1s[:], p1g.ap())
            p2s = cpool.tile([128, C2 // 16], I16)
            nc.sync.dma_start(p2s[:], p2g.ap())

            nc.gpsimd.load_library(library_config.mlp)

            for _rep in range(reps):
                run_rep(nc, tc, pool, psum, h2pool, dpool, xt, y,
                        w1s, w2s, w345s, b1s, b2s, p1s, p2s,
                        nb, b345, relu_t, sigm_t, ablate)

    nc.compile()
    return nc


def run_rep(nc, tc, pool, psum, h2pool, dpool, xt, y,
            w1s, w2s, w345s, b1s, b2s, p1s, p2s,
            nb, b345, relu_t, sigm_t, ablate=0):
    h1d = dpool.tile([C2, nb], BF16, tag="h1d", name="h1d", bufs=2)

    # ---- phase B: GL1 (DRAM gather of xt + paired matmuls -> h1d) ----
    for k in range(C1 // K1):
        g1 = pool.tile([128, K1 // 128, nb], BF16, tag="g1", name="g1", bufs=2)
        nc.gpsimd.dma_gather(
            g1[:], xt.ap(), p1s[:, k * (K1 // 16):(k + 1) * (K1 // 16)],
            K1, K1, nb, single_packet=False, queue_num=k % 4)
        if ablate == 3:
            continue
        h1st = None
        for j in range(K1 // 256):          # pairs of groups
            pair = (K1 // 256) * k + j
            ps_b = psum.tile([128, nb], F32, tag="ps_b")
            for half in range(2):
                blk = 2 * j + half
                g = (K1 // 128) * k + blk
                nc.tensor.matmul(
                    ps_b[64 * half:64 * (half + 1), :],
                    lhsT=w1s[:, g * 64:(g + 1) * 64], rhs=g1[:, blk, :],
                    start=True, stop=True,
                    tile_position=(0, 64 * half) if half else None)
            if j % 2 == 0:
                h1st = pool.tile([128, 2 * nb], BF16, tag="h1st", bufs=3,
                                 name="h1st")
            nc.scalar.activation(h1st[:, (j % 2) * nb:(j % 2 + 1) * nb],
                                 ps_b[:], relu_t,
                                 bias=b1s[:, pair:pair + 1])
            if j % 2 == 1:
                ch0 = (pair - 1) * 128
                weng = nc.sync if (pair // 2) % 2 == 0 else nc.scalar
                weng.dma_start(
                    h1d[ch0:ch0 + 256, :].rearrange(
                        "(c p) b -> p c b", c=2), h1st[:])

    if ablate == 3:
        yt0 = pool.tile([1, nb], F32, tag="yt", bufs=2, name="yt0")
        nc.vector.tensor_copy(yt0[:], g1[0:1, 0, :])
        nc.sync.dma_start(y.ap(), yt0[:])
        return
    if ablate == 4:
        rb = pool.tile([1, nb], BF16, tag="rb", bufs=2, name="rb")
        nc.sync.dma_start(rb[:], h1d[C2 - 1:C2, :])
        yt0 = pool.tile([1, nb], F32, tag="yt", bufs=2, name="yt0")
        nc.vector.tensor_copy(yt0[:], rb[:])
        nc.sync.dma_start(y.ap(), yt0[:])
        return

    # ---- phase C: GL2 (DRAM gather + block-diag matmuls) ----
    h2 = []
    for k in range(C2 // K2):
        g2 = pool.tile([128, K2 // 128, nb], BF16, tag="g2", name="g2")
        nc.gpsimd.dma_gather(
            g2[:], h1d[:], p2s[:, k * (K2 // 16):(k + 1) * (K2 // 16)],
            K2, K2, nb, single_packet=False, queue_num=k % 4)
        for j in range(K2 // 256):          # quads of groups
            t = (K2 // 256) * k + j
            ps_c = psum.tile([128, nb], F32, tag="ps_b", bufs=2)
            for half in range(2):
                blk = 2 * j + half
                q = (K2 // 128) * k + blk
                nc.tensor.matmul(
                    ps_c[64 * half:64 * (half + 1), :],
                    lhsT=w2s[:, q * 64:(q + 1) * 64], rhs=g2[:, blk, :],
                    start=True, stop=True,
                    tile_position=(0, 64 * half) if half else None)
            h2t = h2pool.tile([128, nb], BF16, tag=f"h2_{t}", name=f"h2_{t}")
            nc.scalar.activation(h2t[:], ps_c[:], relu_t,
                                 bias=b2s[:, t:t + 1])
            h2.append(h2t)

    # ---- phase D: head ----
    ps_d = psum.tile([1, nb], F32, tag="ps_d", bufs=1)
    for t in range(32):
        nc.tensor.matmul(ps_d[:], lhsT=w345s[:, t:t + 1], rhs=h2[t][:],
                         start=(t == 0), stop=(t == 31))
    yt = pool.tile([1, nb], F32, tag="yt", bufs=2)
    nc.scalar.activation(yt[:], ps_d[:], sigm_t, bias=float(b345))
    nc.sync.dma_start(y.ap(), yt[:])


def _gather_table(perm: np.ndarray, chunk: int) -> np.ndarray:
    """Index table for dma_gather: chunk c occupies cols [c*chunk/16,
    (c+1)*chunk/16); within a chunk, idx position i = col*16 + row.
    Rows 0-15 hold the indices; replicated to all 128 partitions."""
    n = perm.shape[0]
    cols = chunk // 16
    t = np.zeros((128, (n // chunk) * cols), dtype=np.int16)
    for c in range(n // chunk):
        blk = perm[c * chunk:(c + 1) * chunk].reshape(cols, 16).T  # [16, cols]
        t[:, c * cols:(c + 1) * cols] = np.tile(blk, (8, 1))
    return t


def prep_host(perm1, perm2, W1, b1, W2, b2, W3, b3, W4, b4, W5, b5,
              mid_bf16=1):
    """Host-side layout prep of weights / index tables (replicated per core)."""
    import ml_dtypes
    wdt = ml_dtypes.bfloat16
    w1h = np.ascontiguousarray(
        W1.astype(np.float32).transpose(1, 0, 2).reshape(128, G * 64)).astype(wdt)
    w2h = np.zeros((128, 64 * 64), dtype=wdt)
    for q in range(64):
        w2h[0:64, q * 64:q * 64 + 32] = W2[2 * q].astype(wdt)
        w2h[64:128, q * 64 + 32:(q + 1) * 64] = W2[2 * q + 1].astype(wdt)
    wv = (W3.astype(np.float64) @ W4.astype(np.float64) @ W5.astype(np.float64))
    w345h = np.ascontiguousarray(
        wv.astype(np.float32).reshape(32, 128).T).astype(wdt)
    b345 = float(
        (b3.astype(np.float64) @ W4.astype(np.float64) @ W5.astype(np.float64)
         + b4.astype(np.float64) @ W5.astype(np.float64)
         + b5.astype(np.float64)).reshape(()))
    b1h = np.ascontiguousarray(
        b1.astype(np.float32).reshape(64, 128).T)   # col k = [b1[2k]; b1[2k+1]]
    b2h = np.ascontiguousarray(
        b2.astype(np.float32).reshape(32, 128).T)   # col j = b2[4j:4j+4] stacked
    p1h = _gather_table(perm1.astype(np.int64), K1)
    p2h = _gather_table(perm2.astype(np.int64), K2)
    return {"w1": w1h, "w2": w2h, "w345": w345h, "b1p": b1h, "b2q": b2h,
            "p1g": p1h, "p2g": p2h}, b345


def prep_x(x_shard: np.ndarray) -> np.ndarray:
    """Host layout prep of the x shard: bf16, channel-major [C1, nb]."""
    import ml_dtypes
    return np.ascontiguousarray(
        x_shard.astype(np.float32).astype(ml_dtypes.bfloat16).T)


_NC_CACHE: dict = {}

MID_BF16 = 1


def get_nc(nb: int, b345: float, key_bytes: bytes, mid_bf16: int = MID_BF16):
    key = (nb, mid_bf16,
           hashlib.sha256(key_bytes + np.float64(b345).tobytes()).hexdigest())
    if key not in _NC_CACHE:
        _NC_CACHE[key] = build_nc(nb, b345, mid_bf16=mid_bf16)
    return _NC_CACHE[key]


def kernel(x, perm1, perm2, W1, b1, W2, b2, W3, b3, W4, b4, W5, b5):
    x = np.asarray(x)
    consts, b345 = prep_host(np.asarray(perm1), np.asarray(perm2),
                             np.asarray(W1), np.asarray(b1), np.asarray(W2),
                             np.asarray(b2), np.asarray(W3), np.asarray(b3),
                             np.asarray(W4), np.asarray(b4), np.asarray(W5),
                             np.asarray(b5), mid_bf16=MID_BF16)
    nb = x.shape[0] // N_CORES
    key_bytes = np.asarray(perm1).tobytes() + np.asarray(perm2).tobytes()
    nc = get_nc(nb, b345, key_bytes)
    in_maps = []
    for c in range(N_CORES):
        m = dict(consts)
        m["xt"] = prep_x(x[c * nb:(c + 1) * nb])
        in_maps.append(m)
    res = bass_utils.run_bass_kernel_spmd(nc, in_maps, core_ids=list(range(N_CORES)))
    out = np.concatenate([res.results[c]["y"].reshape(nb) for c in range(N_CORES)])
    return out.reshape(-1, 1).astype(np.float32)
